# revision 4
# baseline (speedup 1.0000x reference)
"""TRN2 Bass kernel for Conv4Pim_group_arr_v2 (LSQ-quantized 3x3 conv, p/n split).

Strategy:
  - Host (numpy, exact fp32 replication of the jax reference):
      * LSQ weight quantization for both branches -> wq fp32 [1024,112,3,3]
        (p-branch = channels 0..511, n-branch = 512..1023)
      * grad_scale'd psum steps sg_p/sg_n and their reciprocals
      * weight layout [ic, oc_tile, pos, m] for PE lhsT tiles
  - Device (8 NeuronCores, data-parallel over batch, 2 images/core):
      * conv as 9 shifted matmuls (f32r, K=112, M=128, N=464) accumulated in
        PSUM over a zero-padded 58x58 image layout
      * psum quantize: ACT magic-round (Copy(ps*inv_sg + 1.5*2^23)), DVE clip
        in magic domain, DVE (sub magic, mul sg), GPSIMD p-n subtract
      * strided DMA extracts the 56x56 interior
"""

import sys

import numpy as np

for _p in ("/opt/trn_rl_repo", "/root/.axon_site/_ro/trn_rl_repo"):
    if _p not in sys.path:
        sys.path.append(_p)

# ---------------- problem constants (hardcoded from the module config) ----
W_BIT, SPLIT_BIT, IDX, PS_BIT = 4, 2, 1, 8
OC, IC, KS, N_ARR = 512, 112, 3, 256
NUM_IC = 28
NUM_OC = 256
ROW, COL = 2, 4          # 2 x 4 sub-arrays
QP_W = 15
QN_PS, QP_PS = -128, 127
SHIFT, BASE = 4, 4
NB, H, W = 16, 56, 56
NCORES = 8
PER_CORE = NB // NCORES   # 2 images per core

PADW = 58                 # padded row width/height
FLAT = PADW * PADW        # 3364
XIMG = 3368               # 1 guard col + 3364 + 3 slack
ROWT = 8                  # padded rows per matmul tile
NT = ROWT * PADW          # 464 matmul free size
RT = 7                    # row tiles per image (rows 1..56)
OCT = 8                   # oc tiles of 128 over 1024 concat channels
WCOLS = OCT * 9 * 128     # 9216
MAGIC = float(np.float32(12582912.0))  # 1.5 * 2**23

_CACHE = {}


# ---------------- host-side exact fp32 quantization ----------------------
def _grad_scale_fwd(s, g32):
    s = np.float32(s)
    t1 = np.float32(s * g32)
    t2 = np.float32(s - t1)
    return np.float32(t1 + t2)


def _quant_weight_branch(w_sign, s_arr):
    """Exact fp32 replication of reference quant_weight forward pass."""
    t = w_sign.reshape(ROW, NUM_OC, COL, NUM_IC, KS, KS).transpose(0, 2, 1, 3, 4, 5)
    tile_size = NUM_OC * NUM_IC * KS * KS
    g32 = np.float32(1.0 / np.sqrt(np.float64(tile_size * QP_W)))
    q = np.empty_like(t)
    s_rc = s_arr.reshape(ROW, COL)
    for r in range(ROW):
        for c in range(COL):
            sg = _grad_scale_fwd(s_rc[r, c], g32)
            d = t[r, c] / sg                      # fp32 division
            cl = np.clip(d, np.float32(0.0), np.float32(QP_W))
            xi = np.rint(cl)                      # RNE, fp32
            sl = np.mod(np.floor(xi / np.float32(SHIFT)), np.float32(BASE))
            q[r, c] = sl * sg                     # fp32 mult
    return q.transpose(0, 2, 1, 3, 4, 5).reshape(OC, IC, KS, KS)


def _host_prepare(weight, sw_p, sw_n, sp_p, sp_n):
    w = np.ascontiguousarray(weight, dtype=np.float32)
    wq_p = _quant_weight_branch(np.maximum(w, np.float32(0.0)),
                                np.asarray(sw_p, np.float32))
    wq_n = _quant_weight_branch(np.maximum(-w, np.float32(0.0)),
                                np.asarray(sw_n, np.float32))
    wq = np.concatenate([wq_p, wq_n], axis=0)            # [1024,112,3,3]
    # [ic, t, pos, m] lhsT layout
    w_host = np.ascontiguousarray(
        wq.reshape(OCT, 128, IC, 9).transpose(2, 0, 3, 1)).reshape(IC, WCOLS)

    g_ps = np.float32(1.0 / np.sqrt(np.float64(NB * OC * H * W) * QP_PS))
    sg_p = _grad_scale_fwd(np.float32(sp_p), g_ps)
    sg_n = _grad_scale_fwd(np.float32(sp_n), g_ps)
    sc = np.zeros((128, 8), np.float32)
    sc[:, 0] = np.float32(1.0 / np.float64(sg_p))
    sc[:, 1] = sg_p
    sc[:, 2] = np.float32(1.0 / np.float64(sg_n))
    sc[:, 3] = sg_n
    return w_host, sc


# ---------------- device program ----------------------------------------
def _build():
    import concourse.bacc as bacc
    import concourse.tile as tile
    from concourse import mybir

    f32 = mybir.dt.float32
    Alu = mybir.AluOpType
    Act = mybir.ActivationFunctionType

    nc = bacc.Bacc("TRN2", target_bir_lowering=False, debug=False)
    x_d = nc.dram_tensor("x", [IC, PER_CORE * XIMG], f32, kind="ExternalInput").ap()
    w_d = nc.dram_tensor("w", [IC, WCOLS], f32, kind="ExternalInput").ap()
    sc_d = nc.dram_tensor("sc", [128, 8], f32, kind="ExternalInput").ap()
    o_d = nc.dram_tensor("out", [PER_CORE, OC, RT, NT], f32,
                         kind="ExternalOutput").ap()

    with tile.TileContext(nc) as tc:
        with (
            tc.tile_pool(name="wq", bufs=1) as wpool,
            tc.tile_pool(name="xbuf", bufs=1) as xbpool,
            tc.tile_pool(name="scp", bufs=1) as scpool,
            tc.tile_pool(name="psum", bufs=8, space="PSUM") as pspool,
            tc.tile_pool(name="y", bufs=6) as ypool,
            tc.tile_pool(name="c", bufs=6) as cpool,
            tc.tile_pool(name="v", bufs=6) as vpool,
            tc.tile_pool(name="o", bufs=4) as opool,
        ):
            sct = scpool.tile([128, 8], f32)
            nc.sync.dma_start(sct[:], sc_d)

            # input: host-padded, contiguous DMA per image
            xb = xbpool.tile([IC, PER_CORE * XIMG], f32)
            for img in range(PER_CORE):
                base = img * XIMG
                nc.sync.dma_start(xb[:, base:base + XIMG],
                                  x_d[:, base:base + XIMG])

            # weights: DMA chunks in the order oc-tiles are consumed
            wrt = wpool.tile([IC, WCOLS], f32)
            CH = 9 * 128  # one oc-tile worth of columns
            for t in (0, 4, 1, 5, 2, 6, 3, 7):
                nc.sync.dma_start(wrt[:, t * CH:(t + 1) * CH],
                                  w_d[:, t * CH:(t + 1) * CH])

            for img in range(PER_CORE):
                for j in range(RT):
                    p0 = img * XIMG + 1 + PADW * (1 + ROWT * j)
                    vtiles = []
                    for pair in range(4):
                        for br, t, sci in ((0, pair, 0), (1, pair + 4, 2)):
                            ps = pspool.tile([128, NT], f32, tag="ps")
                            for pos in range(9):
                                dy, dx = pos // 3 - 1, pos % 3 - 1
                                roff = p0 + dy * PADW + dx
                                nc.tensor.matmul(
                                    ps[:],
                                    wrt[:, t * CH + pos * 128: t * CH + (pos + 1) * 128],
                                    xb[:, roff: roff + NT],
                                    start=(pos == 0), stop=(pos == 8),
                                )
                            y = ypool.tile([128, NT], f32, tag="y")
                            nc.scalar.activation(y[:], ps[:], Act.Copy,
                                                 bias=MAGIC,
                                                 scale=sct[:, sci:sci + 1])
                            c = cpool.tile([128, NT], f32, tag="c")
                            nc.vector.tensor_scalar(
                                c[:], y[:],
                                float(np.float32(MAGIC) + np.float32(QN_PS)),
                                float(np.float32(MAGIC) + np.float32(QP_PS)),
                                Alu.max, Alu.min)
                            v = vpool.tile([128, NT], f32, tag="v")
                            nc.vector.tensor_scalar(
                                v[:], c[:], MAGIC, sct[:, sci + 1:sci + 2],
                                Alu.subtract, Alu.mult)
                            vtiles.append(v)
                        vp, vn = vtiles[-2], vtiles[-1]
                        o = opool.tile([128, NT], f32, tag="o")
                        nc.gpsimd.tensor_tensor(o[:], vp[:], vn[:], Alu.subtract)
                        nc.sync.dma_start(
                            o_d[img, pair * 128:(pair + 1) * 128, j, :], o[:])

    nc.compile()
    return nc


def _get_program():
    if "nc" not in _CACHE:
        _CACHE["nc"] = _build()
    return _CACHE["nc"]


def kernel(input, weight, sw_p, sw_n, sp_p, sp_n):
    from concourse import bass_utils

    x = np.ascontiguousarray(np.asarray(input, np.float32))
    w_host, sc = _host_prepare(np.asarray(weight, np.float32),
                               sw_p, sw_n, sp_p, sp_n)

    nc = _get_program()
    in_maps = []
    for cidx in range(NCORES):
        xp = np.zeros((IC, PER_CORE * XIMG), np.float32)
        for img in range(PER_CORE):
            base = img * XIMG
            view = xp[:, base + 60: base + 60 + PADW * H]
            view.reshape(IC, H, PADW)[:, :, 0:W] = \
                x[cidx * PER_CORE + img]
        in_maps.append({"x": xp, "w": w_host, "sc": sc})

    res = bass_utils.run_bass_kernel_spmd(nc, in_maps,
                                          core_ids=list(range(NCORES)))
    out = np.empty((NB, OC, H, W), np.float32)
    for c in range(NCORES):
        op = res.results[c]["out"].reshape(PER_CORE, OC, RT, ROWT, PADW)
        out[c * PER_CORE:(c + 1) * PER_CORE] = \
            op[:, :, :, :, 1:57].reshape(PER_CORE, OC, H, W)
    return out


# revision 6
# speedup vs baseline: 1.7927x; 1.7927x over previous
"""TRN2 Bass kernel for Conv4Pim_group_arr_v2 (LSQ-quantized 3x3 conv, p/n split).

Strategy:
  - Host (numpy, exact fp32 replication of the jax reference):
      * LSQ weight quantization for both branches -> wq fp32 [1024,112,3,3]
        (p-branch = channels 0..511, n-branch = 512..1023)
      * grad_scale'd psum steps sg_p/sg_n and their reciprocals
      * weight layout [ic, oc_tile, pos, m] for PE lhsT tiles
  - Device (8 NeuronCores, data-parallel over batch, 2 images/core):
      * conv as 9 shifted matmuls (f32r, K=112, M=128, N=464) accumulated in
        PSUM over a zero-padded 58x58 image layout
      * psum quantize: ACT magic-round (Copy(ps*inv_sg + 1.5*2^23)), DVE clip
        in magic domain, DVE (sub magic, mul sg), GPSIMD p-n subtract
      * strided DMA extracts the 56x56 interior
"""

import sys

import numpy as np

for _p in ("/opt/trn_rl_repo", "/root/.axon_site/_ro/trn_rl_repo"):
    if _p not in sys.path:
        sys.path.append(_p)

# ---------------- problem constants (hardcoded from the module config) ----
W_BIT, SPLIT_BIT, IDX, PS_BIT = 4, 2, 1, 8
OC, IC, KS, N_ARR = 512, 112, 3, 256
NUM_IC = 28
NUM_OC = 256
ROW, COL = 2, 4          # 2 x 4 sub-arrays
QP_W = 15
QN_PS, QP_PS = -128, 127
SHIFT, BASE = 4, 4
NB, H, W = 16, 56, 56
NCORES = 8
PER_CORE = NB // NCORES   # 2 images per core

PADW = 58                 # padded row width/height
FLAT = PADW * PADW        # 3364
XIMG = 3368               # 1 guard col + 3364 + 3 slack
ROWT = 8                  # padded rows per matmul tile
NT = ROWT * PADW          # 464 matmul free size
RT = 7                    # row tiles per image (rows 1..56)
OCT = 8                   # oc tiles of 128 over 1024 concat channels
WCOLS = OCT * 9 * 128     # 9216
MAGIC = float(np.float32(12582912.0))  # 1.5 * 2**23

_CACHE = {}


# ---------------- host-side exact fp32 quantization ----------------------
def _grad_scale_fwd(s, g32):
    s = np.float32(s)
    t1 = np.float32(s * g32)
    t2 = np.float32(s - t1)
    return np.float32(t1 + t2)


def _quant_digits_branch(w_sign, s_arr):
    """Exact fp32 replication of reference quant_weight forward pass,
    returning integer digit levels (0..3) and the per-(row,col) grad-scaled
    steps separately (digits are exact in bf16; steps get folded into x)."""
    t = w_sign.reshape(ROW, NUM_OC, COL, NUM_IC, KS, KS).transpose(0, 2, 1, 3, 4, 5)
    tile_size = NUM_OC * NUM_IC * KS * KS
    g32 = np.float32(1.0 / np.sqrt(np.float64(tile_size * QP_W)))
    dig = np.empty_like(t)
    sg_rc = np.empty((ROW, COL), np.float32)
    s_rc = s_arr.reshape(ROW, COL)
    for r in range(ROW):
        for c in range(COL):
            sg = _grad_scale_fwd(s_rc[r, c], g32)
            sg_rc[r, c] = sg
            d = t[r, c] / sg                      # fp32 division
            cl = np.clip(d, np.float32(0.0), np.float32(QP_W))
            xi = np.rint(cl)                      # RNE, fp32
            dig[r, c] = np.mod(np.floor(xi / np.float32(SHIFT)), np.float32(BASE))
    return (dig.transpose(0, 2, 1, 3, 4, 5).reshape(OC, IC, KS, KS), sg_rc)


def _host_prepare(weight, sw_p, sw_n, sp_p, sp_n):
    import ml_dtypes
    w = np.ascontiguousarray(weight, dtype=np.float32)
    dig_p, sg_w_p = _quant_digits_branch(np.maximum(w, np.float32(0.0)),
                                         np.asarray(sw_p, np.float32))
    dig_n, sg_w_n = _quant_digits_branch(np.maximum(-w, np.float32(0.0)),
                                         np.asarray(sw_n, np.float32))
    dig = np.concatenate([dig_p, dig_n], axis=0)         # [1024,112,3,3]
    # [ic, t, pos, m] lhsT layout; digits {0..3} are exact in bf16
    w_host = np.ascontiguousarray(
        dig.reshape(OCT, 128, IC, 9).transpose(2, 0, 3, 1)
    ).reshape(IC, WCOLS).astype(ml_dtypes.bfloat16)
    # per-(branch,row) per-partition weight-step vectors: svec[v][ic],
    # v = 2*branch + row, step chosen by ic//NUM_IC column block
    svec = np.empty((4, IC), np.float32)
    for b, sgw in ((0, sg_w_p), (1, sg_w_n)):
        for r in range(ROW):
            svec[2 * b + r] = np.repeat(sgw[r], NUM_IC)

    g_ps = np.float32(1.0 / np.sqrt(np.float64(NB * OC * H * W) * QP_PS))
    sg_p = _grad_scale_fwd(np.float32(sp_p), g_ps)
    sg_n = _grad_scale_fwd(np.float32(sp_n), g_ps)
    sc = np.zeros((128, 8), np.float32)
    sc[:, 0] = np.float32(1.0 / np.float64(sg_p))
    sc[:, 1] = sg_p
    sc[:, 2] = np.float32(1.0 / np.float64(sg_n))
    sc[:, 3] = sg_n
    return w_host, sc, svec


# ---------------- device program ----------------------------------------
def _build():
    import concourse.bacc as bacc
    import concourse.tile as tile
    from concourse import mybir

    f32 = mybir.dt.float32
    bf16 = mybir.dt.bfloat16
    Alu = mybir.AluOpType
    Act = mybir.ActivationFunctionType

    nc = bacc.Bacc("TRN2", target_bir_lowering=False, debug=False)
    xh_d = nc.dram_tensor("xh", [4, IC, PER_CORE * XIMG], bf16,
                          kind="ExternalInput").ap()
    xl_d = nc.dram_tensor("xl", [4, IC, PER_CORE * XIMG], bf16,
                          kind="ExternalInput").ap()
    w_d = nc.dram_tensor("w", [IC, WCOLS], bf16, kind="ExternalInput").ap()
    sc_d = nc.dram_tensor("sc", [128, 8], f32, kind="ExternalInput").ap()
    o_d = nc.dram_tensor("out", [PER_CORE, OC, RT, NT], f32,
                         kind="ExternalOutput").ap()

    with tile.TileContext(nc) as tc:
        with (
            tc.tile_pool(name="wq", bufs=1) as wpool,
            tc.tile_pool(name="xbuf", bufs=1) as xbpool,
            tc.tile_pool(name="scp", bufs=1) as scpool,
            tc.tile_pool(name="psum", bufs=8, space="PSUM") as pspool,
            tc.tile_pool(name="y", bufs=6) as ypool,
            tc.tile_pool(name="c", bufs=6) as cpool,
            tc.tile_pool(name="v", bufs=6) as vpool,
            tc.tile_pool(name="o", bufs=4) as opool,
        ):
            sct = scpool.tile([128, 8], f32)
            nc.sync.dma_start(sct[:], sc_d)

            # input: host-padded scaled hi/lo bf16 copies, 4 variants
            # v = 2*branch + oc_row; contiguous DMA per (variant, image)
            CI = PER_CORE * XIMG
            xhb = xbpool.tile([IC, 4 * CI], bf16, tag="xh")
            xlb = xbpool.tile([IC, 4 * CI], bf16, tag="xl")
            for img in range(PER_CORE):
                for v in range(4):
                    base = img * XIMG
                    nc.sync.dma_start(
                        xhb[:, v * CI + base: v * CI + base + XIMG],
                        xh_d[v, :, base:base + XIMG])
                    nc.sync.dma_start(
                        xlb[:, v * CI + base: v * CI + base + XIMG],
                        xl_d[v, :, base:base + XIMG])

            # weights (digit levels, exact bf16): chunks in consumption order
            wrt = wpool.tile([IC, WCOLS], bf16)
            CH = 9 * 128  # one oc-tile worth of columns
            for t in (0, 4, 1, 5, 2, 6, 3, 7):
                nc.sync.dma_start(wrt[:, t * CH:(t + 1) * CH],
                                  w_d[:, t * CH:(t + 1) * CH])

            for img in range(PER_CORE):
                for j in range(RT):
                    p0 = img * XIMG + 1 + PADW * (1 + ROWT * j)
                    vtiles = []
                    for pair in range(4):
                        for br, t, sci in ((0, pair, 0), (1, pair + 4, 2)):
                            v = 2 * br + (pair // 2)
                            ps = pspool.tile([128, NT], f32, tag="ps")
                            for pos in range(9):
                                dy, dx = pos // 3 - 1, pos % 3 - 1
                                roff = v * CI + p0 + dy * PADW + dx
                                wsl = wrt[:, t * CH + pos * 128:
                                          t * CH + (pos + 1) * 128]
                                nc.tensor.matmul(
                                    ps[:], wsl, xhb[:, roff: roff + NT],
                                    start=(pos == 0), stop=False)
                                nc.tensor.matmul(
                                    ps[:], wsl, xlb[:, roff: roff + NT],
                                    start=False, stop=(pos == 8))
                            y = ypool.tile([128, NT], f32, tag="y")
                            nc.scalar.activation(y[:], ps[:], Act.Copy,
                                                 bias=MAGIC,
                                                 scale=sct[:, sci:sci + 1])
                            c = cpool.tile([128, NT], f32, tag="c")
                            nc.vector.tensor_scalar(
                                c[:], y[:],
                                float(np.float32(MAGIC) + np.float32(QN_PS)),
                                float(np.float32(MAGIC) + np.float32(QP_PS)),
                                Alu.max, Alu.min)
                            v = vpool.tile([128, NT], f32, tag="v")
                            nc.vector.tensor_scalar(
                                v[:], c[:], MAGIC, sct[:, sci + 1:sci + 2],
                                Alu.subtract, Alu.mult)
                            vtiles.append(v)
                        vp, vn = vtiles[-2], vtiles[-1]
                        o = opool.tile([128, NT], f32, tag="o")
                        nc.gpsimd.tensor_tensor(o[:], vp[:], vn[:], Alu.subtract)
                        nc.sync.dma_start(
                            o_d[img, pair * 128:(pair + 1) * 128, j, :], o[:])

    nc.compile()
    return nc


def _get_program():
    if "nc" not in _CACHE:
        _CACHE["nc"] = _build()
    return _CACHE["nc"]


def _marshal_x(x_core, svec):
    """Pad both images, scale by the 4 (branch,row) step vectors, split
    into bf16 hi + lo."""
    import ml_dtypes
    xp = np.zeros((IC, PER_CORE * XIMG), np.float32)
    for img in range(PER_CORE):
        base = img * XIMG
        view = xp[:, base + 60: base + 60 + PADW * H]
        view.reshape(IC, H, PADW)[:, :, 0:W] = x_core[img]
    xs = xp[None, :, :] * svec[:, :, None]        # [4, IC, cols] fp32
    xh = xs.astype(ml_dtypes.bfloat16)
    xl = (xs - xh.astype(np.float32)).astype(ml_dtypes.bfloat16)
    return np.ascontiguousarray(xh), np.ascontiguousarray(xl)


def kernel(input, weight, sw_p, sw_n, sp_p, sp_n):
    from concourse import bass_utils

    x = np.ascontiguousarray(np.asarray(input, np.float32))
    w_host, sc, svec = _host_prepare(np.asarray(weight, np.float32),
                                     sw_p, sw_n, sp_p, sp_n)

    nc = _get_program()
    in_maps = []
    for cidx in range(NCORES):
        xh, xl = _marshal_x(x[cidx * PER_CORE:(cidx + 1) * PER_CORE], svec)
        in_maps.append({"xh": xh, "xl": xl, "w": w_host, "sc": sc})

    res = bass_utils.run_bass_kernel_spmd(nc, in_maps,
                                          core_ids=list(range(NCORES)))
    out = np.empty((NB, OC, H, W), np.float32)
    for c in range(NCORES):
        op = res.results[c]["out"].reshape(PER_CORE, OC, RT, ROWT, PADW)
        out[c * PER_CORE:(c + 1) * PER_CORE] = \
            op[:, :, :, :, 1:57].reshape(PER_CORE, OC, H, W)
    return out


# revision 7
# speedup vs baseline: 1.9037x; 1.0619x over previous
"""TRN2 Bass kernel for Conv4Pim_group_arr_v2 (LSQ-quantized 3x3 conv, p/n split).

Strategy:
  - Host (numpy, exact fp32 replication of the jax reference):
      * LSQ weight quantization for both branches -> wq fp32 [1024,112,3,3]
        (p-branch = channels 0..511, n-branch = 512..1023)
      * grad_scale'd psum steps sg_p/sg_n and their reciprocals
      * weight layout [ic, oc_tile, pos, m] for PE lhsT tiles
  - Device (8 NeuronCores, data-parallel over batch, 2 images/core):
      * conv as 9 shifted matmuls (f32r, K=112, M=128, N=464) accumulated in
        PSUM over a zero-padded 58x58 image layout
      * psum quantize: ACT magic-round (Copy(ps*inv_sg + 1.5*2^23)), DVE clip
        in magic domain, DVE (sub magic, mul sg), GPSIMD p-n subtract
      * strided DMA extracts the 56x56 interior
"""

import sys

import numpy as np

for _p in ("/opt/trn_rl_repo", "/root/.axon_site/_ro/trn_rl_repo"):
    if _p not in sys.path:
        sys.path.append(_p)

# ---------------- problem constants (hardcoded from the module config) ----
W_BIT, SPLIT_BIT, IDX, PS_BIT = 4, 2, 1, 8
OC, IC, KS, N_ARR = 512, 112, 3, 256
NUM_IC = 28
NUM_OC = 256
ROW, COL = 2, 4          # 2 x 4 sub-arrays
QP_W = 15
QN_PS, QP_PS = -128, 127
SHIFT, BASE = 4, 4
NB, H, W = 16, 56, 56
NCORES = 8
PER_CORE = NB // NCORES   # 2 images per core

PADW = 58                 # padded row width/height
FLAT = PADW * PADW        # 3364
XIMG = 3368               # 1 guard col + 3364 + 3 slack
ROWT = 8                  # padded rows per matmul tile
NT = ROWT * PADW          # 464 matmul free size
RT = 7                    # row tiles per image (rows 1..56)
OCT = 8                   # oc tiles of 128 over 1024 concat channels
WCOLS = OCT * 9 * 128     # 9216
MAGIC = float(np.float32(12582912.0))  # 1.5 * 2**23

_CACHE = {}


# ---------------- host-side exact fp32 quantization ----------------------
def _grad_scale_fwd(s, g32):
    s = np.float32(s)
    t1 = np.float32(s * g32)
    t2 = np.float32(s - t1)
    return np.float32(t1 + t2)


def _quant_digits_branch(w_sign, s_arr):
    """Exact fp32 replication of reference quant_weight forward pass,
    returning integer digit levels (0..3) and the per-(row,col) grad-scaled
    steps separately (digits are exact in bf16; steps get folded into x)."""
    t = w_sign.reshape(ROW, NUM_OC, COL, NUM_IC, KS, KS).transpose(0, 2, 1, 3, 4, 5)
    tile_size = NUM_OC * NUM_IC * KS * KS
    g32 = np.float32(1.0 / np.sqrt(np.float64(tile_size * QP_W)))
    dig = np.empty_like(t)
    sg_rc = np.empty((ROW, COL), np.float32)
    s_rc = s_arr.reshape(ROW, COL)
    for r in range(ROW):
        for c in range(COL):
            sg = _grad_scale_fwd(s_rc[r, c], g32)
            sg_rc[r, c] = sg
            d = t[r, c] / sg                      # fp32 division
            cl = np.clip(d, np.float32(0.0), np.float32(QP_W))
            xi = np.rint(cl)                      # RNE, fp32
            dig[r, c] = np.mod(np.floor(xi / np.float32(SHIFT)), np.float32(BASE))
    return (dig.transpose(0, 2, 1, 3, 4, 5).reshape(OC, IC, KS, KS), sg_rc)


def _host_prepare(weight, sw_p, sw_n, sp_p, sp_n):
    import ml_dtypes
    w = np.ascontiguousarray(weight, dtype=np.float32)
    dig_p, sg_w_p = _quant_digits_branch(np.maximum(w, np.float32(0.0)),
                                         np.asarray(sw_p, np.float32))
    dig_n, sg_w_n = _quant_digits_branch(np.maximum(-w, np.float32(0.0)),
                                         np.asarray(sw_n, np.float32))
    dig = np.concatenate([dig_p, dig_n], axis=0)         # [1024,112,3,3]
    # [ic, t, pos, m] lhsT layout; digits {0..3} are exact in bf16
    w_host = np.ascontiguousarray(
        dig.reshape(OCT, 128, IC, 9).transpose(2, 0, 3, 1)
    ).reshape(IC, WCOLS).astype(ml_dtypes.bfloat16)
    # per-(branch,row) per-partition weight-step vectors: svec[v][ic],
    # v = 2*branch + row, step chosen by ic//NUM_IC column block
    svec = np.empty((4, IC), np.float32)
    for b, sgw in ((0, sg_w_p), (1, sg_w_n)):
        for r in range(ROW):
            svec[2 * b + r] = np.repeat(sgw[r], NUM_IC)

    g_ps = np.float32(1.0 / np.sqrt(np.float64(NB * OC * H * W) * QP_PS))
    sg_p = _grad_scale_fwd(np.float32(sp_p), g_ps)
    sg_n = _grad_scale_fwd(np.float32(sp_n), g_ps)
    sc = np.zeros((128, 8), np.float32)
    sc[:, 0] = np.float32(1.0 / np.float64(sg_p))
    sc[:, 1] = sg_p
    sc[:, 2] = np.float32(1.0 / np.float64(sg_n))
    sc[:, 3] = sg_n
    return w_host, sc, svec


# ---------------- device program ----------------------------------------
def _build():
    import concourse.bacc as bacc
    import concourse.tile as tile
    from concourse import mybir

    f32 = mybir.dt.float32
    bf16 = mybir.dt.bfloat16
    Alu = mybir.AluOpType
    Act = mybir.ActivationFunctionType

    nc = bacc.Bacc("TRN2", target_bir_lowering=False, debug=False)
    xh_d = nc.dram_tensor("xh", [4, IC, PER_CORE * XIMG], bf16,
                          kind="ExternalInput").ap()
    xl_d = nc.dram_tensor("xl", [4, IC, PER_CORE * XIMG], bf16,
                          kind="ExternalInput").ap()
    w_d = nc.dram_tensor("w", [IC, WCOLS], bf16, kind="ExternalInput").ap()
    sc_d = nc.dram_tensor("sc", [128, 8], f32, kind="ExternalInput").ap()
    o_d = nc.dram_tensor("out", [PER_CORE, OC, RT, NT], f32,
                         kind="ExternalOutput").ap()

    with tile.TileContext(nc) as tc:
        with (
            tc.tile_pool(name="wq", bufs=1) as wpool,
            tc.tile_pool(name="xbuf", bufs=1) as xbpool,
            tc.tile_pool(name="scp", bufs=1) as scpool,
            tc.tile_pool(name="psum", bufs=8, space="PSUM") as pspool,
            tc.tile_pool(name="y", bufs=6) as ypool,
            tc.tile_pool(name="c", bufs=6) as cpool,
            tc.tile_pool(name="v", bufs=6) as vpool,
            tc.tile_pool(name="o", bufs=4) as opool,
        ):
            sct = scpool.tile([128, 8], f32)
            nc.sync.dma_start(sct[:], sc_d)

            # input: host-padded scaled hi/lo bf16 copies, 4 variants
            # v = 2*branch + oc_row; contiguous DMA per (variant, image)
            CI = PER_CORE * XIMG
            xhb = xbpool.tile([IC, 4 * CI], bf16, tag="xh")
            xlb = xbpool.tile([IC, 4 * CI], bf16, tag="xl")
            wrt = wpool.tile([IC, WCOLS], bf16)
            CH = 9 * 128  # one oc-tile worth of columns

            def dma_x(v, img):
                base = img * XIMG
                nc.sync.dma_start(
                    xhb[:, v * CI + base: v * CI + base + XIMG],
                    xh_d[v, :, base:base + XIMG])
                nc.sync.dma_start(
                    xlb[:, v * CI + base: v * CI + base + XIMG],
                    xl_d[v, :, base:base + XIMG])

            def dma_w(t):
                nc.sync.dma_start(wrt[:, t * CH:(t + 1) * CH],
                                  w_d[:, t * CH:(t + 1) * CH])

            # first psum group needs (v=0, img0) + w[t0]; emit in the exact
            # order the compute loop consumes so PE starts ASAP
            dma_x(0, 0); dma_w(0)
            dma_x(2, 0); dma_w(4)
            dma_x(1, 0); dma_w(1); dma_w(5)
            dma_x(3, 0); dma_w(2); dma_w(6); dma_w(3); dma_w(7)
            for v in (0, 2, 1, 3):
                dma_x(v, 1)

            for img in range(PER_CORE):
                for j in range(RT):
                    p0 = img * XIMG + 1 + PADW * (1 + ROWT * j)
                    vtiles = []
                    for pair in range(4):
                        for br, t, sci in ((0, pair, 0), (1, pair + 4, 2)):
                            v = 2 * br + (pair // 2)
                            ps = pspool.tile([128, NT], f32, tag="ps")
                            for pos in range(9):
                                dy, dx = pos // 3 - 1, pos % 3 - 1
                                roff = v * CI + p0 + dy * PADW + dx
                                wsl = wrt[:, t * CH + pos * 128:
                                          t * CH + (pos + 1) * 128]
                                nc.tensor.matmul(
                                    ps[:], wsl, xhb[:, roff: roff + NT],
                                    start=(pos == 0), stop=False)
                                nc.tensor.matmul(
                                    ps[:], wsl, xlb[:, roff: roff + NT],
                                    start=False, stop=(pos == 8))
                            y = ypool.tile([128, NT], f32, tag="y")
                            nc.scalar.activation(y[:], ps[:], Act.Copy,
                                                 bias=MAGIC,
                                                 scale=sct[:, sci:sci + 1])
                            c = cpool.tile([128, NT], f32, tag="c")
                            nc.vector.tensor_scalar(
                                c[:], y[:],
                                float(np.float32(MAGIC) + np.float32(QN_PS)),
                                float(np.float32(MAGIC) + np.float32(QP_PS)),
                                Alu.max, Alu.min)
                            v = vpool.tile([128, NT], f32, tag="v")
                            nc.vector.tensor_scalar(
                                v[:], c[:], MAGIC, sct[:, sci + 1:sci + 2],
                                Alu.subtract, Alu.mult)
                            vtiles.append(v)
                        vp, vn = vtiles[-2], vtiles[-1]
                        o = opool.tile([128, NT], f32, tag="o")
                        nc.gpsimd.tensor_tensor(o[:], vp[:], vn[:], Alu.subtract)
                        nc.sync.dma_start(
                            o_d[img, pair * 128:(pair + 1) * 128, j, :], o[:])

    nc.compile()
    return nc


def _get_program():
    if "nc" not in _CACHE:
        _CACHE["nc"] = _build()
    return _CACHE["nc"]


def _marshal_x(x_core, svec):
    """Pad both images, scale by the 4 (branch,row) step vectors, split
    into bf16 hi + lo."""
    import ml_dtypes
    xp = np.zeros((IC, PER_CORE * XIMG), np.float32)
    for img in range(PER_CORE):
        base = img * XIMG
        view = xp[:, base + 60: base + 60 + PADW * H]
        view.reshape(IC, H, PADW)[:, :, 0:W] = x_core[img]
    xs = xp[None, :, :] * svec[:, :, None]        # [4, IC, cols] fp32
    xh = xs.astype(ml_dtypes.bfloat16)
    xl = (xs - xh.astype(np.float32)).astype(ml_dtypes.bfloat16)
    return np.ascontiguousarray(xh), np.ascontiguousarray(xl)


def kernel(input, weight, sw_p, sw_n, sp_p, sp_n):
    from concourse import bass_utils

    x = np.ascontiguousarray(np.asarray(input, np.float32))
    w_host, sc, svec = _host_prepare(np.asarray(weight, np.float32),
                                     sw_p, sw_n, sp_p, sp_n)

    nc = _get_program()
    in_maps = []
    for cidx in range(NCORES):
        xh, xl = _marshal_x(x[cidx * PER_CORE:(cidx + 1) * PER_CORE], svec)
        in_maps.append({"xh": xh, "xl": xl, "w": w_host, "sc": sc})

    res = bass_utils.run_bass_kernel_spmd(nc, in_maps,
                                          core_ids=list(range(NCORES)))
    out = np.empty((NB, OC, H, W), np.float32)
    for c in range(NCORES):
        op = res.results[c]["out"].reshape(PER_CORE, OC, RT, ROWT, PADW)
        out[c * PER_CORE:(c + 1) * PER_CORE] = \
            op[:, :, :, :, 1:57].reshape(PER_CORE, OC, H, W)
    return out
